# revision 25
# baseline (speedup 1.0000x reference)
"""GQA attention (int8-quantized QK^T, RoPE, causal softmax) on 8 TRN2 NeuronCores.

Sharding: tensor-parallel over heads. Core c owns Q heads 4c..4c+3 (Wq cols
512c..512c+512), KV head c (Wk/Wv cols 128c..128c+128), and Wo rows
512c..512c+512. x is replicated. Each core emits a partial [2048, 4096]
output (its heads' contribution through Wo); the host sums the 8 partials.
No on-device collectives.

Block-interleaved schedule: for each 512-row block b, emit
  - x^T DMA-transposes for block b (Sync queue, runs behind PE),
  - A(b): Q/K/V projections + RoPE + absmax-quantize + PE transposes,
  - B(J=b): attention q-block b against all t-chunks <= b (causal),
  - C(b): output-projection rows of block b + partial-sum DMA out,
so TensorE sees one dense stream and phases overlap instead of serializing.
QK^T is exact: int-quantized values stored in bf16 (ints <= 127 are exact),
accumulated in fp32 PSUM.
"""

import numpy as np

import concourse.bass as bass
import concourse.mybir as mybir
import concourse.tile as tile
from concourse import bacc
from concourse.bass_utils import run_bass_kernel_spmd
from concourse.masks import make_identity

FP = mybir.dt.float32
BF = mybir.dt.bfloat16
AL = mybir.AluOpType
AF = mybir.ActivationFunctionType

B, S, D, NH, NKV, HD = 1, 2048, 4096, 32, 8, 128
NCORES = 8
HPC = NH // NCORES          # 4 Q heads per core
FQ = HPC * HD               # 512
SCALE = HD ** -0.5
MAGIC = 3 * 2.0 ** 22       # fp32 round-to-nearest-even magic constant

ST = S // 128               # 16 s-tiles of 128 rows
DC = D // 128               # 32 d-chunks
SBLK = 4                    # s-tiles per block (512 rows); block b == q-block J
NB = ST // SBLK             # 4 blocks


def build_graph():
    nc = bacc.Bacc(None)
    x_e = nc.declare_dram_parameter("x", [S, D], FP, isOutput=False)
    wq_e = nc.declare_dram_parameter("wq", [D, FQ], FP, isOutput=False)
    wk_e = nc.declare_dram_parameter("wk", [D, HD], FP, isOutput=False)
    wv_e = nc.declare_dram_parameter("wv", [D, HD], FP, isOutput=False)
    wo_e = nc.declare_dram_parameter("wo", [FQ, D], FP, isOutput=False)
    cos_e = nc.declare_dram_parameter("cos", [S, HD], FP, isOutput=False)
    sin_e = nc.declare_dram_parameter("sin", [S, HD], FP, isOutput=False)
    out_e = nc.declare_dram_parameter("out", [S, D], FP, isOutput=True)

    with tile.TileContext(nc) as tc:
        with (
            tc.tile_pool(name="persist", bufs=1) as pp,
            tc.tile_pool(name="psum", bufs=1, space="PSUM") as psu,
            tc.tile_pool(name="blkp", bufs=2) as bp,
            tc.tile_pool(name="xtp", bufs=1) as xtp,
            tc.tile_pool(name="dram", bufs=1, space="DRAM") as drp,
            tc.tile_pool(name="ab", bufs=2) as ab,
            tc.tile_pool(name="att", bufs=3) as at,
            tc.tile_pool(name="attf", bufs=4) as atf,
            tc.tile_pool(name="ost", bufs=2) as ost,
        ):
            ident = pp.tile([128, 128], BF)
            make_identity(nc, ident[:])
            idf = pp.tile([128, 128], FP)
            make_identity(nc, idf[:])
            ones1 = pp.tile([128, 1], BF)       # den stationary (M=1)
            nc.gpsimd.memset(ones1[:], 1.0)

            kT = pp.tile([128, S], BF)          # quantized K^T (persist: causal reuse)
            vn = pp.tile([128, ST, HD], BF)     # V natural, per t-chunk (persist)
            ksr = pp.tile([128, ST], FP)        # k dequant scale (SCALE folded)

            # SWDGE queue order == emission order: x block 0 first (unblocks
            # the transpose pipeline), then Wq (first projection), Wkv, Wo,
            # then the rest of x.
            xdrs = []
            for blk in range(NB):
                xdr = drp.tile([SBLK * 128, D], BF, tag=f"xdr{blk}")
                xdrs.append(xdr)
            nc.gpsimd.dma_start(xdrs[0][:], x_e[0:SBLK * 128, :])
            wqr = pp.tile([128, DC, FQ], BF)
            for wc in range(4):
                nc.gpsimd.dma_start(
                    wqr[:, wc * 8:(wc + 1) * 8, :],
                    wq_e[:].rearrange("(c p) f -> p c f", p=128)[:, wc * 8:(wc + 1) * 8, :])
            wkv = pp.tile([128, DC, 2 * HD], BF)
            nc.gpsimd.dma_start(wkv[:, :, 0:HD], wk_e[:].rearrange("(c p) h -> p c h", p=128))
            nc.gpsimd.dma_start(wkv[:, :, HD:2 * HD], wv_e[:].rearrange("(c p) h -> p c h", p=128))
            wo_r = pp.tile([128, HPC, D], BF)
            nc.gpsimd.dma_start(wo_r[:], wo_e[:].rearrange("(f p) d -> p f d", p=128))
            for blk in range(1, NB):
                r0 = blk * SBLK * 128
                nc.gpsimd.dma_start(xdrs[blk][:], x_e[r0:r0 + SBLK * 128, :])

            cosr = pp.tile([128, ST, HD], BF)
            sinm = pp.tile([128, ST, HD], BF)    # [-sin | +sin] halves
            nc.gpsimd.dma_start(cosr[:], cos_e[:].rearrange("(t p) d -> p t d", p=128))
            nc.gpsimd.dma_start(sinm[:], sin_e[:].rearrange("(t p) d -> p t d", p=128))
            nc.vector.tensor_scalar_mul(sinm[:, :, 0:64], sinm[:, :, 0:64], -1.0)

            for blk in range(NB):
                J = blk
                xdr = xdrs[blk]
                # ---- x^T for this block via DMA-transpose (Sync queue)
                xTs = []
                for d in range(DC):
                    xTd = xtp.tile([128, SBLK * 128], BF, tag=f"xT{d}")
                    xTs.append(xTd)
                # fence absorbs the xdr-writer dep into the Sync queue clock
                fen = xtp.tile([1, 32], BF, tag="fence", bufs=2)
                nc.sync.dma_start(fen[:], xdr[0:1, 0:32])
                for d in range(DC):
                    nc.sync.dma_start(xTs[d][:],
                                      xdr[:, d * 128:(d + 1) * 128],
                                      transpose=True)

                # per-block products of phase A consumed by B(blk)/C(blk)
                qTb = bp.tile([128, HPC, SBLK * 128], BF, tag="qTb")
                qsrTb = bp.tile([1, HPC, SBLK * 128], BF, tag="qsrTb")
                OTb = bp.tile([128, HPC, SBLK * 128], BF, tag="OTb")

                # ---- A(blk): projections + RoPE + quantize
                for i in range(SBLK):
                    st_i = blk * SBLK + i
                    q_ps = psu.tile([128, FQ], FP, tag="big", bufs=3)
                    kv_ps = psu.tile([128, 2 * HD], FP, tag="o", bufs=2)
                    for d in range(DC):
                        nc.tensor.matmul(q_ps[:], xTs[d][:, i * 128:(i + 1) * 128],
                                         wqr[:, d, :],
                                         start=(d == 0), stop=(d == DC - 1))
                    for d in range(DC):
                        nc.tensor.matmul(kv_ps[:], xTs[d][:, i * 128:(i + 1) * 128],
                                         wkv[:, d, :],
                                         start=(d == 0), stop=(d == DC - 1))

                    nc.scalar.copy(vn[:, st_i, :], kv_ps[:, HD:2 * HD])

                    qi = ab.tile([128, FQ], BF, tag="qi")
                    ki = ab.tile([128, HD], BF, tag="ki")
                    for (src, nh, i8out) in ((q_ps, HPC, qi), (kv_ps, 1, ki)):
                        rr = ab.tile([128, nh, HD], FP, tag=f"rr{nh}")
                        t2 = ab.tile([128, nh, HD], FP, tag=f"t2{nh}")
                        am = ab.tile([128, nh], FP, tag=f"am{nh}")
                        sc = ab.tile([128, nh], FP, tag=f"sc{nh}")
                        for h in range(nh):
                            co = cosr[:, st_i, :]
                            si = sinm[:, st_i, :]
                            nc.vector.tensor_mul(rr[:, h, :], src[:, h * HD:(h + 1) * HD], co)
                            nc.vector.tensor_mul(t2[:, h, 0:64], src[:, h * HD + 64:(h + 1) * HD], si[:, 0:64])
                            nc.vector.tensor_mul(t2[:, h, 64:HD], src[:, h * HD:h * HD + 64], si[:, 64:HD])
                        nc.vector.tensor_add(rr[:], rr[:], t2[:])
                        nc.vector.tensor_reduce(am[:], rr[:], axis=mybir.AxisListType.X,
                                                op=AL.max, apply_absolute_value=True)
                        nc.vector.tensor_scalar_max(am[:], am[:], 1e-5)
                        nc.vector.reciprocal_approx_fast(sc[:], am[:])   # ~1/amax
                        for h in range(nh):
                            nc.vector.tensor_scalar(rr[:, h, :], rr[:, h, :],
                                                    sc[:, h:h + 1], None, op0=AL.mult)
                        nc.vector.tensor_scalar(rr[:], rr[:], 127.0, MAGIC, op0=AL.mult, op1=AL.add)
                        nc.vector.tensor_scalar(i8out[:], rr[:], MAGIC, None, op0=AL.subtract)
                        if nh == 1:
                            nc.vector.tensor_scalar_mul(ksr[:, st_i:st_i + 1], am[:], SCALE / 127.0)
                        else:
                            nc.vector.tensor_scalar_mul(am[:], am[:], 1.0 / 127.0)
                            for h in range(HPC):
                                qsr_ps = psu.tile([1, 128], FP, tag="tp", bufs=2)
                                nc.tensor.transpose(qsr_ps[:], am[:, h:h + 1], idf[:])
                                nc.scalar.copy(qsrTb[0:1, h, i * 128:(i + 1) * 128],
                                               qsr_ps[:])

                    for h in range(HPC):
                        tp = psu.tile([128, 128], BF, tag="tp", bufs=2)
                        nc.tensor.transpose(tp[:], qi[:, h * HD:(h + 1) * HD], ident[:])
                        nc.scalar.copy(qTb[:, h, i * 128:(i + 1) * 128], tp[:])
                    tp = psu.tile([128, 128], BF, tag="tp", bufs=2)
                    nc.tensor.transpose(tp[:], ki[:], ident[:])
                    nc.scalar.copy(kT[:, st_i * 128:(st_i + 1) * 128], tp[:])

                # ---- B(J=blk): attention for q rows [J*512, J*512+512)
                nlive = 4 * J + 4
                for h in range(HPC):
                    dqb = at.tile([128, 512], BF, tag="dqb")
                    nc.gpsimd.partition_broadcast(dqb[:], qsrTb[0:1, h, :])
                    dq = at.tile([128, 512], FP, tag="dq")
                    nc.scalar.copy(dq[:], dqb[:])

                    oT_ps = psu.tile([128, 512], FP, tag="o", bufs=2)
                    den_ps = psu.tile([1, 512], FP, tag="den", bufs=1)
                    for ti in range(nlive):
                        sc_ps = psu.tile([128, 512], FP, tag="big", bufs=3)
                        nc.tensor.matmul(sc_ps[:], kT[:, ti * 128:(ti + 1) * 128],
                                         qTb[:, h, :])
                        ptf = atf.tile([128, 512], FP, tag="ptf")
                        nc.vector.scalar_tensor_tensor(
                            out=ptf[:], in0=sc_ps[:], scalar=ksr[:, ti:ti + 1],
                            in1=dq[:], op0=AL.mult, op1=AL.mult)
                        pt = atf.tile([128, 512], BF, tag="pt")
                        nc.scalar.activation(pt[:], ptf[:], AF.Exp)
                        if ti >= 4 * J:
                            nc.gpsimd.affine_select(
                                out=pt[:], in_=pt[:],
                                compare_op=AL.is_ge, fill=0.0,
                                base=J * 512 - ti * 128, channel_multiplier=-1,
                                pattern=[[1, 512]])
                        nc.tensor.matmul(den_ps[:], ones1[:], pt[:],
                                         start=(ti == 0), stop=(ti == nlive - 1))
                        nc.tensor.matmul(oT_ps[:], vn[:, ti, :], pt[:],
                                         start=(ti == 0), stop=(ti == nlive - 1))

                    denr = at.tile([1, 512], FP, tag="denr")
                    nc.vector.reciprocal_approx_fast(denr[:], den_ps[:])
                    dnb = at.tile([128, 512], FP, tag="dnb")
                    nc.gpsimd.partition_broadcast(dnb[:], denr[:])
                    nc.vector.tensor_mul(OTb[:, h, :], oT_ps[:], dnb[:])

                # ---- C(blk): output projection rows of this block
                for i in range(SBLK):
                    for half in range(4):
                        ot_sb = ost.tile([128, D // 4], FP, tag="ot")
                        for dbl in range(2):
                            db = half * 2 + dbl
                            wo_ps = psu.tile([128, 512], FP, tag="big", bufs=3)
                            for f in range(HPC):
                                nc.tensor.matmul(wo_ps[:], OTb[:, f, i * 128:(i + 1) * 128],
                                                 wo_r[:, f, db * 512:(db + 1) * 512],
                                                 start=(f == 0), stop=(f == HPC - 1))
                            if db % 2 == 0:
                                nc.scalar.copy(ot_sb[:, dbl * 512:(dbl + 1) * 512], wo_ps[:])
                            else:
                                nc.vector.tensor_copy(ot_sb[:, dbl * 512:(dbl + 1) * 512], wo_ps[:])
                        st_i = blk * SBLK + i
                        nc.sync.dma_start(
                            out_e[st_i * 128:(st_i + 1) * 128,
                                  half * (D // 4):(half + 1) * (D // 4)],
                            ot_sb[:])

    nc.compile()
    return nc


_CACHE = {}


def kernel(x, Wq, Wk, Wv, Wo, cos, sin):
    x2 = np.ascontiguousarray(np.asarray(x, np.float32).reshape(S, D))
    in_maps = []
    for c in range(NCORES):
        in_maps.append({
            "x": x2,
            "wq": np.ascontiguousarray(Wq[:, c * FQ:(c + 1) * FQ], np.float32),
            "wk": np.ascontiguousarray(Wk[:, c * HD:(c + 1) * HD], np.float32),
            "wv": np.ascontiguousarray(Wv[:, c * HD:(c + 1) * HD], np.float32),
            "wo": np.ascontiguousarray(Wo[c * FQ:(c + 1) * FQ, :], np.float32),
            "cos": np.ascontiguousarray(cos, np.float32),
            "sin": np.ascontiguousarray(sin, np.float32),
        })
    if "nc" not in _CACHE:
        _CACHE["nc"] = build_graph()
    res = run_bass_kernel_spmd(_CACHE["nc"], in_maps, core_ids=list(range(NCORES)))
    out = np.zeros((S, D), np.float64)
    for r in res.results:
        out += np.asarray(r["out"], np.float64)
    return out.astype(np.float32).reshape(B, S, D)


# revision 26
# speedup vs baseline: 1.1183x; 1.1183x over previous
"""GQA attention (int8-quantized QK^T, RoPE, causal softmax) on 8 TRN2 NeuronCores.

Sharding: tensor-parallel over heads. Core c owns Q heads 4c..4c+3 (Wq cols
512c..512c+512), KV head c (Wk/Wv cols 128c..128c+128), and Wo rows
512c..512c+512. x is replicated. Each core emits a partial [2048, 4096]
output (its heads' contribution through Wo); the host sums the 8 partials.
No on-device collectives.

Block-interleaved schedule: for each 512-row block b, emit
  - x^T DMA-transposes for block b (Sync queue, runs behind PE),
  - A(b): Q/K/V projections + RoPE + absmax-quantize + PE transposes,
  - B(J=b): attention q-block b against all t-chunks <= b (causal),
  - C(b): output-projection rows of block b + partial-sum DMA out,
so TensorE sees one dense stream and phases overlap instead of serializing.
QK^T is exact: int-quantized values stored in bf16 (ints <= 127 are exact),
accumulated in fp32 PSUM.
"""

import numpy as np

import concourse.bass as bass
import concourse.mybir as mybir
import concourse.tile as tile
from concourse import bacc
from concourse.bass_utils import run_bass_kernel_spmd
from concourse.masks import make_identity

FP = mybir.dt.float32
BF = mybir.dt.bfloat16
AL = mybir.AluOpType
AF = mybir.ActivationFunctionType

B, S, D, NH, NKV, HD = 1, 2048, 4096, 32, 8, 128
NCORES = 8
HPC = NH // NCORES          # 4 Q heads per core
FQ = HPC * HD               # 512
SCALE = HD ** -0.5
MAGIC = 3 * 2.0 ** 22       # fp32 round-to-nearest-even magic constant

ST = S // 128               # 16 s-tiles of 128 rows
DC = D // 128               # 32 d-chunks
SBLK = 4                    # s-tiles per block (512 rows); block b == q-block J
NB = ST // SBLK             # 4 blocks


def build_graph():
    nc = bacc.Bacc(None)
    x_e = nc.declare_dram_parameter("x", [S, D], FP, isOutput=False)
    wq_e = nc.declare_dram_parameter("wq", [D, FQ], FP, isOutput=False)
    wk_e = nc.declare_dram_parameter("wk", [D, HD], FP, isOutput=False)
    wv_e = nc.declare_dram_parameter("wv", [D, HD], FP, isOutput=False)
    wo_e = nc.declare_dram_parameter("wo", [FQ, D], FP, isOutput=False)
    cos_e = nc.declare_dram_parameter("cos", [S, HD], FP, isOutput=False)
    sin_e = nc.declare_dram_parameter("sin", [S, HD], FP, isOutput=False)
    out_e = nc.declare_dram_parameter("out", [S, D], FP, isOutput=True)

    with tile.TileContext(nc) as tc:
        with (
            tc.tile_pool(name="persist", bufs=1) as pp,
            tc.tile_pool(name="psum", bufs=1, space="PSUM") as psu,
            tc.tile_pool(name="blkp", bufs=2) as bp,
            tc.tile_pool(name="xtp", bufs=1) as xtp,
            tc.tile_pool(name="dram", bufs=1, space="DRAM") as drp,
            tc.tile_pool(name="ab", bufs=2) as ab,
            tc.tile_pool(name="att", bufs=3) as at,
            tc.tile_pool(name="attf", bufs=4) as atf,
            tc.tile_pool(name="ost", bufs=2) as ost,
        ):
            ident = pp.tile([128, 128], BF)
            make_identity(nc, ident[:])
            idf = pp.tile([128, 128], FP)
            make_identity(nc, idf[:])
            ones1 = pp.tile([128, 1], BF)       # den stationary (M=1)
            nc.gpsimd.memset(ones1[:], 1.0)

            kT = pp.tile([128, S], BF)          # quantized K^T (persist: causal reuse)
            vn = pp.tile([128, ST, HD], BF)     # V natural, per t-chunk (persist)
            ksr = pp.tile([128, ST], FP)        # k dequant scale (SCALE folded)

            # SWDGE queue order == emission order: x block 0 first (unblocks
            # the transpose pipeline), then Wq (first projection), Wkv, Wo,
            # then the rest of x.
            xdrs = []
            for blk in range(NB):
                xdr = drp.tile([SBLK * 128, D], BF, tag=f"xdr{blk}")
                xdrs.append(xdr)
            def cast_x(blk):
                r0 = blk * SBLK * 128
                nc.gpsimd.dma_start(xdrs[blk][:], x_e[r0:r0 + SBLK * 128, :])
            cast_x(0)
            wqr = pp.tile([128, DC, FQ], BF)
            for wc in range(4):
                nc.gpsimd.dma_start(
                    wqr[:, wc * 8:(wc + 1) * 8, :],
                    wq_e[:].rearrange("(c p) f -> p c f", p=128)[:, wc * 8:(wc + 1) * 8, :])
            cosr = pp.tile([128, ST, HD], BF)
            sinm = pp.tile([128, ST, HD], BF)    # [-sin | +sin] halves
            nc.gpsimd.dma_start(cosr[:], cos_e[:].rearrange("(t p) d -> p t d", p=128))
            nc.gpsimd.dma_start(sinm[:], sin_e[:].rearrange("(t p) d -> p t d", p=128))
            nc.vector.tensor_scalar_mul(sinm[:, :, 0:64], sinm[:, :, 0:64], -1.0)
            cast_x(1)
            wkv = pp.tile([128, DC, 2 * HD], BF)
            nc.gpsimd.dma_start(wkv[:, :, 0:HD], wk_e[:].rearrange("(c p) h -> p c h", p=128))
            nc.gpsimd.dma_start(wkv[:, :, HD:2 * HD], wv_e[:].rearrange("(c p) h -> p c h", p=128))
            cast_x(2)
            wo_r = pp.tile([128, HPC, D], BF)
            nc.gpsimd.dma_start(wo_r[:], wo_e[:].rearrange("(f p) d -> p f d", p=128))
            cast_x(3)

            for blk in range(NB):
                J = blk
                xdr = xdrs[blk]
                # ---- x^T for this block via DMA-transpose (Sync queue)
                xTs = []
                for d in range(DC):
                    xTd = xtp.tile([128, SBLK * 128], BF, tag=f"xT{d}")
                    xTs.append(xTd)
                for d in range(DC):
                    nc.sync.dma_start(xTs[d][:],
                                      xdr[:, d * 128:(d + 1) * 128],
                                      transpose=True)

                # per-block products of phase A consumed by B(blk)/C(blk)
                qTb = bp.tile([128, HPC, SBLK * 128], BF, tag="qTb")
                qsrTb = bp.tile([1, HPC, SBLK * 128], BF, tag="qsrTb")
                OTb = bp.tile([128, HPC, SBLK * 128], BF, tag="OTb")

                # ---- A(blk): projections + RoPE + quantize
                for i in range(SBLK):
                    st_i = blk * SBLK + i
                    q_ps = psu.tile([128, FQ], FP, tag="big", bufs=3)
                    kv_ps = psu.tile([128, 2 * HD], FP, tag="o", bufs=2)
                    for d in range(DC):
                        nc.tensor.matmul(q_ps[:], xTs[d][:, i * 128:(i + 1) * 128],
                                         wqr[:, d, :],
                                         start=(d == 0), stop=(d == DC - 1))
                    for d in range(DC):
                        nc.tensor.matmul(kv_ps[:], xTs[d][:, i * 128:(i + 1) * 128],
                                         wkv[:, d, :],
                                         start=(d == 0), stop=(d == DC - 1))

                    nc.scalar.copy(vn[:, st_i, :], kv_ps[:, HD:2 * HD])

                    qi = ab.tile([128, FQ], BF, tag="qi")
                    ki = ab.tile([128, HD], BF, tag="ki")
                    for (src, nh, i8out) in ((q_ps, HPC, qi), (kv_ps, 1, ki)):
                        rr = ab.tile([128, nh, HD], FP, tag=f"rr{nh}")
                        t2 = ab.tile([128, nh, HD], FP, tag=f"t2{nh}")
                        am = ab.tile([128, nh], FP, tag=f"am{nh}")
                        sc = ab.tile([128, nh], FP, tag=f"sc{nh}")
                        for h in range(nh):
                            co = cosr[:, st_i, :]
                            si = sinm[:, st_i, :]
                            nc.vector.tensor_mul(rr[:, h, :], src[:, h * HD:(h + 1) * HD], co)
                            nc.vector.tensor_mul(t2[:, h, 0:64], src[:, h * HD + 64:(h + 1) * HD], si[:, 0:64])
                            nc.vector.tensor_mul(t2[:, h, 64:HD], src[:, h * HD:h * HD + 64], si[:, 64:HD])
                        nc.vector.tensor_add(rr[:], rr[:], t2[:])
                        nc.vector.tensor_reduce(am[:], rr[:], axis=mybir.AxisListType.X,
                                                op=AL.max, apply_absolute_value=True)
                        nc.vector.tensor_scalar_max(am[:], am[:], 1e-5)
                        nc.vector.reciprocal_approx_fast(sc[:], am[:])   # ~1/amax
                        for h in range(nh):
                            nc.vector.tensor_scalar(rr[:, h, :], rr[:, h, :],
                                                    sc[:, h:h + 1], None, op0=AL.mult)
                        nc.vector.tensor_scalar(rr[:], rr[:], 127.0, MAGIC, op0=AL.mult, op1=AL.add)
                        nc.vector.tensor_scalar(i8out[:], rr[:], MAGIC, None, op0=AL.subtract)
                        if nh == 1:
                            nc.vector.tensor_scalar_mul(ksr[:, st_i:st_i + 1], am[:], SCALE / 127.0)
                        else:
                            nc.vector.tensor_scalar_mul(am[:], am[:], 1.0 / 127.0)
                            for h in range(HPC):
                                qsr_ps = psu.tile([1, 128], FP, tag="tp", bufs=2)
                                nc.tensor.transpose(qsr_ps[:], am[:, h:h + 1], idf[:])
                                nc.scalar.copy(qsrTb[0:1, h, i * 128:(i + 1) * 128],
                                               qsr_ps[:])

                    for h in range(HPC):
                        tp = psu.tile([128, 128], BF, tag="tp", bufs=2)
                        nc.tensor.transpose(tp[:], qi[:, h * HD:(h + 1) * HD], ident[:])
                        nc.scalar.copy(qTb[:, h, i * 128:(i + 1) * 128], tp[:])
                    tp = psu.tile([128, 128], BF, tag="tp", bufs=2)
                    nc.tensor.transpose(tp[:], ki[:], ident[:])
                    nc.scalar.copy(kT[:, st_i * 128:(st_i + 1) * 128], tp[:])

                # ---- B(J=blk): attention for q rows [J*512, J*512+512)
                nlive = 4 * J + 4
                for h in range(HPC):
                    dqb = at.tile([128, 512], BF, tag="dqb")
                    nc.gpsimd.partition_broadcast(dqb[:], qsrTb[0:1, h, :])
                    dq = at.tile([128, 512], FP, tag="dq")
                    nc.scalar.copy(dq[:], dqb[:])

                    oT_ps = psu.tile([128, 512], FP, tag="o", bufs=2)
                    den_ps = psu.tile([1, 512], FP, tag="den", bufs=1)
                    for ti in range(nlive):
                        sc_ps = psu.tile([128, 512], FP, tag="big", bufs=3)
                        nc.tensor.matmul(sc_ps[:], kT[:, ti * 128:(ti + 1) * 128],
                                         qTb[:, h, :])
                        ptf = atf.tile([128, 512], FP, tag="ptf")
                        nc.vector.scalar_tensor_tensor(
                            out=ptf[:], in0=sc_ps[:], scalar=ksr[:, ti:ti + 1],
                            in1=dq[:], op0=AL.mult, op1=AL.mult)
                        pt = atf.tile([128, 512], BF, tag="pt")
                        nc.scalar.activation(pt[:], ptf[:], AF.Exp)
                        if ti >= 4 * J:
                            nc.gpsimd.affine_select(
                                out=pt[:], in_=pt[:],
                                compare_op=AL.is_ge, fill=0.0,
                                base=J * 512 - ti * 128, channel_multiplier=-1,
                                pattern=[[1, 512]])
                        nc.tensor.matmul(den_ps[:], ones1[:], pt[:],
                                         start=(ti == 0), stop=(ti == nlive - 1))
                        nc.tensor.matmul(oT_ps[:], vn[:, ti, :], pt[:],
                                         start=(ti == 0), stop=(ti == nlive - 1))

                    denr = at.tile([1, 512], FP, tag="denr")
                    nc.vector.reciprocal_approx_fast(denr[:], den_ps[:])
                    dnb = at.tile([128, 512], FP, tag="dnb")
                    nc.gpsimd.partition_broadcast(dnb[:], denr[:])
                    nc.vector.tensor_mul(OTb[:, h, :], oT_ps[:], dnb[:])

                # ---- C(blk): output projection rows of this block
                for i in range(SBLK):
                    for half in range(4):
                        ot_sb = ost.tile([128, D // 4], FP, tag="ot")
                        for dbl in range(2):
                            db = half * 2 + dbl
                            wo_ps = psu.tile([128, 512], FP, tag="big", bufs=3)
                            for f in range(HPC):
                                nc.tensor.matmul(wo_ps[:], OTb[:, f, i * 128:(i + 1) * 128],
                                                 wo_r[:, f, db * 512:(db + 1) * 512],
                                                 start=(f == 0), stop=(f == HPC - 1))
                            if db % 2 == 0:
                                nc.scalar.copy(ot_sb[:, dbl * 512:(dbl + 1) * 512], wo_ps[:])
                            else:
                                nc.vector.tensor_copy(ot_sb[:, dbl * 512:(dbl + 1) * 512], wo_ps[:])
                        st_i = blk * SBLK + i
                        nc.scalar.dma_start(
                            out_e[st_i * 128:(st_i + 1) * 128,
                                  half * (D // 4):(half + 1) * (D // 4)],
                            ot_sb[:])

    nc.compile()
    return nc


_CACHE = {}


def kernel(x, Wq, Wk, Wv, Wo, cos, sin):
    x2 = np.ascontiguousarray(np.asarray(x, np.float32).reshape(S, D))
    in_maps = []
    for c in range(NCORES):
        in_maps.append({
            "x": x2,
            "wq": np.ascontiguousarray(Wq[:, c * FQ:(c + 1) * FQ], np.float32),
            "wk": np.ascontiguousarray(Wk[:, c * HD:(c + 1) * HD], np.float32),
            "wv": np.ascontiguousarray(Wv[:, c * HD:(c + 1) * HD], np.float32),
            "wo": np.ascontiguousarray(Wo[c * FQ:(c + 1) * FQ, :], np.float32),
            "cos": np.ascontiguousarray(cos, np.float32),
            "sin": np.ascontiguousarray(sin, np.float32),
        })
    if "nc" not in _CACHE:
        _CACHE["nc"] = build_graph()
    res = run_bass_kernel_spmd(_CACHE["nc"], in_maps, core_ids=list(range(NCORES)))
    out = np.zeros((S, D), np.float64)
    for r in res.results:
        out += np.asarray(r["out"], np.float64)
    return out.astype(np.float32).reshape(B, S, D)
